# revision 1
# baseline (speedup 1.0000x reference)
"""Trainium2 Bass kernel: MHSA with multi-head relative position embedding.

Sharding: data-parallel over batch — 16 batches / 8 cores = 2 batches per core,
each core computes all 8 heads for its 2 batches. No collectives needed.

Math per batch (N=784 tokens, C=512, 8 heads x 64 dim):
  qkv = x @ w_qkv                  (q-columns pre-scaled by 1/8 on host)
  scores_T[k,q] = k_h^T q_h        (full row [112,784] in a 2-bank PSUM tile;
                                    even/odd head matmuls issued adjacently so
                                    they pack into disjoint PE row groups)
  E = exp(scores_T) * expbias_T    (exp on ACT in one 784-wide op; bias
                                    multiply on DVE/GPSIMD, bias exp'd on host)
  O_T[d,q] = sum_k v_aug[k,d] E[k,q]  with v_aug = [v | 1] -> row 64 = sumexp
  attnT = O_T[0:64] / O_T[64]      (denominator rows DMA-gathered, recip'd in
                                    half-batch groups, replicated to 128
                                    partitions with a stride-0 DMA, DVE mult)
  out = attnT^T stacked over heads @ w_out   (written bf16, host casts f32)

Schedule: b-major (all 4 head pairs of batch 0, then batch 1) so batch 0's
output projection overlaps batch 1's attention. Projection work (qkv feature
tiles, v tiles, out-proj token tiles) is spread as fillers across both
batches' attention kt-loops to keep the PE warm (HAM at full clock) without
overloading any single phase.
"""

import numpy as np
import ml_dtypes

B, HH, WW, C = 16, 28, 28, 512
N = HH * WW            # 784 tokens
HEADS, KD = 8, 64
NCORES, BPC = 8, 2     # 8 cores, 2 batches per core
NT, TP = 7, 112        # 784 = 7 tiles of 112 (k / token tiling)
CHUNKS = [(0, 512), (512, 272)]   # q-chunks (PSUM bank = 512 fp32)
CT = 4                 # contraction tiles of 128 over C=512

_CACHE = {}


def _rel_index():
    # Faithful to reference._relative_position_index: token r -> (r%28, r//28)
    t = np.arange(N)
    c0, c1 = t % HH, t // HH
    return ((c0[:, None] - c0[None, :] + HH - 1)
            + (c1[:, None] - c1[None, :] + WW - 1) * (2 * HH - 1))  # [q, k]


def build_nc():
    if 'nc' in _CACHE:
        return _CACHE['nc']
    from contextlib import ExitStack
    import concourse.bacc as bacc
    import concourse.mybir as mybir
    import concourse.tile as tile
    from concourse.alu_op_type import AluOpType

    f32 = mybir.dt.float32
    bf16 = mybir.dt.bfloat16
    EXP = mybir.ActivationFunctionType.Exp

    nc = bacc.Bacc("TRN2", debug=False, enable_asserts=False)
    xT_d = nc.dram_tensor("xT", [BPC, C, N], bf16, kind="ExternalInput").ap()
    wqkv_d = nc.dram_tensor("wqkv", [C, 3 * C], bf16, kind="ExternalInput").ap()
    wout_d = nc.dram_tensor("wout", [C, C], bf16, kind="ExternalInput").ap()
    bias_d = nc.dram_tensor("biasT", [HEADS, N, N], bf16, kind="ExternalInput").ap()
    out_d = nc.dram_tensor("out", [BPC, N, C], bf16, kind="ExternalOutput").ap()

    with tile.TileContext(nc) as tc, ExitStack() as ctx:
        persist = ctx.enter_context(tc.tile_pool(name="persist", bufs=1))
        xT_pool = ctx.enter_context(tc.tile_pool(name="xTp", bufs=8))
        bias_pool = ctx.enter_context(tc.tile_pool(name="biasp", bufs=24))
        eraw_pool = ctx.enter_context(tc.tile_pool(name="erp", bufs=4))
        e_pool = ctx.enter_context(tc.tile_pool(name="ep", bufs=1))
        attn_pool = ctx.enter_context(tc.tile_pool(name="atp", bufs=8))
        den_pool = ctx.enter_context(tc.tile_pool(name="dnp", bufs=2))
        rb_pool = ctx.enter_context(tc.tile_pool(name="rbp", bufs=3))
        osb_pool = ctx.enter_context(tc.tile_pool(name="osbp", bufs=2))
        sc_psum = ctx.enter_context(tc.tile_pool(name="scp", bufs=1, space="PSUM"))
        o_psum = ctx.enter_context(tc.tile_pool(name="opp", bufs=1, space="PSUM"))
        pj_psum = ctx.enter_context(tc.tile_pool(name="pjp", bufs=2, space="PSUM"))

        # ---- weights resident in SBUF ----
        wqkv_sb, wout_sb = [], []
        for ci in range(CT):
            w = persist.tile([128, 3 * C], bf16, tag=f"wqkv{ci}")
            nc.sync.dma_start(w, wqkv_d[ci * 128:(ci + 1) * 128, :])
            wqkv_sb.append(w)
        def load_wout():
            for ci in range(CT):
                w = persist.tile([128, C], bf16, tag=f"wout{ci}",
                                 name=f"wout{ci}")
                nc.sync.dma_start(w, wout_d[ci * 128:(ci + 1) * 128, :])
                wout_sb.append(w)

        # warm up the ACT exp table load early (overlaps with qkv phase)
        warm = persist.tile([1, 16], f32, tag="warm")
        nc.vector.memset(warm, 0.0)
        nc.scalar.activation(warm, warm, EXP)

        qkT, vsb, attnT, attn_sb, den_bf = {}, {}, {}, {}, {}
        for b in range(BPC):
            for fi in range(CT):
                attnT[b, fi] = persist.tile(
                    [128, N], bf16, tag=f"attnT{b}_{fi}", name=f"attnT{b}_{fi}")
            den_bf[b, 0] = persist.tile([4, N], bf16, tag=f"den{b}_0",
                                        name=f"den{b}_0")
            for pp in (2, 3):
                den_bf[b, 10 + pp] = persist.tile(
                    [2, N], bf16, tag=f"denp{b}_{pp}", name=f"denp{b}_{pp}")

        xts = {}

        def emit_xt(b):
            tiles = []
            for ci in range(CT):
                xt = xT_pool.tile([128, N], bf16, tag="xT", name=f"xT{b}_{ci}")
                nc.sync.dma_start(xt, xT_d[b, ci * 128:(ci + 1) * 128, :])
                tiles.append(xt)
            xts[b] = tiles

        def emit_qk_tile(b, ft):
            dst = persist.tile([128, N], bf16, tag=f"qkT{b}_{ft}",
                               name=f"qkT{b}_{ft}")
            qkT[b, ft] = dst
            for (c0w, cw) in CHUNKS:
                ps = pj_psum.tile([128, 512], f32, tag="pj",
                                  name=f"pj{b}_{ft}_{c0w}")
                for ci in range(CT):
                    nc.tensor.matmul(
                        ps[:, 0:cw], wqkv_sb[ci][:, ft * 128:(ft + 1) * 128],
                        xts[b][ci][:, c0w:c0w + cw],
                        start=(ci == 0), stop=(ci == CT - 1))
                nc.vector.tensor_copy(dst[:, c0w:c0w + cw], ps[:, 0:cw])

        def emit_v_unit(b, t):
            vt = persist.tile([TP, HEADS, KD + 2], bf16, tag=f"v{b}_{t}",
                              name=f"v{b}_{t}")
            vsb[b, t] = vt
            ps = pj_psum.tile([128, 512], f32, tag="pj", name=f"pv{b}_{t}")
            for ci in range(CT):
                nc.tensor.matmul(
                    ps[0:TP, :], xts[b][ci][:, t * TP:(t + 1) * TP],
                    wqkv_sb[ci][:, 2 * C:3 * C],
                    start=(ci == 0), stop=(ci == CT - 1))
            nc.vector.tensor_copy(
                vt[:, :, 0:KD], ps[0:TP, :].rearrange("p (h d) -> p h d", h=HEADS))
            nc.vector.memset(vt[:, :, KD:KD + 2], 1.0)

        def emit_out_unit(b, t):
            ps = pj_psum.tile([128, 512], f32, tag="pj", name=f"po{b}_{t}")
            for fi in range(CT):
                nc.tensor.matmul(
                    ps[0:TP, :], attnT[b, fi][:, t * TP:(t + 1) * TP],
                    wout_sb[fi], start=(fi == 0), stop=(fi == CT - 1))
            osb = osb_pool.tile([TP, C], bf16, tag="osb")
            nc.vector.tensor_copy(osb, ps[0:TP, :])
            nc.sync.dma_start(out_d[b, t * TP:(t + 1) * TP, :], osb)

        def attention(b, pair, fillers=()):
            fillers = list(fillers)
            h0, h1 = 2 * pair, 2 * pair + 1
            streams = ((0, h0), (1, h1))
            with nc.named_scope(f"attn_b{b}_p{pair}"):
                bias_sb = {}
                for h in (h0, h1):
                    for kt in range(NT):
                        bt = bias_pool.tile([TP, N], bf16, tag="bias",
                                            name=f"bias{h}_{kt}")
                        nc.sync.dma_start(bt, bias_d[h, kt * TP:(kt + 1) * TP, :])
                        bias_sb[h, kt] = bt
                kT_t, qT_t = qkT[b, 4 + pair], qkT[b, pair]
                ops0, esbs, att, scs, eraws = {}, {}, {}, {}, {}
                for kt in range(NT):
                    # scores: even/odd head matmuls adjacent -> disjoint PE
                    # row groups (base partition 0 vs 64) run concurrently
                    for hs, h in streams:
                        scs[hs] = sc_psum.tile([TP, 1024], f32, tag=f"sc{hs}",
                                               name=f"sc{b}_{h}_{kt}")
                    for (c0w, cw) in CHUNKS:
                        for hs, h in streams:
                            r0 = (h % 2) * 64
                            nc.tensor.matmul(
                                scs[hs][:, c0w:c0w + cw],
                                kT_t[r0:r0 + 64, kt * TP:(kt + 1) * TP],
                                qT_t[r0:r0 + 64, c0w:c0w + cw],
                                start=True, stop=True)
                    for hs, h in streams:
                        eraws[hs] = eraw_pool.tile([TP, N], bf16, tag="eraw",
                                                   name=f"er{b}_{h}_{kt}")
                        nc.scalar.activation(eraws[hs], scs[hs][:, 0:N], EXP)
                    for hs, h in streams:
                        esb = e_pool.tile([TP, N], bf16, tag=f"e{hs}_{kt}",
                                          name=f"e{b}_{h}_{kt}")
                        eng = nc.gpsimd if kt in (2, 5) else nc.vector
                        eng.tensor_tensor(esb, eraws[hs], bias_sb[h, kt],
                                          AluOpType.mult)
                        esbs[hs, kt] = esb
                    # chunk-0 v-matmul trails by two kt so its bias-multiply
                    # has two iterations of slack (covers the slower GPSIMD
                    # tensor_tensor on the offloaded kts)
                    if kt >= 2:
                        for hs, h in streams:
                            if kt == 2:
                                ops0[hs] = o_psum.tile([KD + 1, 512], f32,
                                                       tag=f"op{hs}",
                                                       name=f"op0_{b}_{h}")
                            nc.tensor.matmul(
                                ops0[hs], vsb[b, kt - 2][:, h, 0:KD + 1],
                                esbs[hs, kt - 2][:, 0:512],
                                start=(kt == 2), stop=False)
                    if fillers:
                        fillers.pop(0)()
                for ktv in (NT - 2, NT - 1):
                    for hs, h in streams:
                        nc.tensor.matmul(
                            ops0[hs], vsb[b, ktv][:, h, 0:KD + 1],
                            esbs[hs, ktv][:, 0:512],
                            start=False, stop=(ktv == NT - 1))
                for hs, h in streams:
                    a = attn_pool.tile([KD + 1, N], bf16, tag="attn",
                                       name=f"attn{b}_{h}")
                    att[hs] = a
                    attn_sb[b, h] = a
                    nc.vector.tensor_copy(a[:, 0:512], ops0[hs])
                for hs, h in streams:
                    # chunk-1 accumulator from the pj pool: decouples these
                    # vMMs from the chunk-0 copy's o_psum slot release, and
                    # frees op{hs} for the next pair one hop earlier
                    ops1 = pj_psum.tile([KD + 1, 512], f32, tag="pj",
                                        name=f"op1_{b}_{h}")
                    for kt in range(NT):
                        nc.tensor.matmul(
                            ops1[:, 0:272], vsb[b, kt][:, h, 0:KD + 1],
                            esbs[hs, kt][:, 512:784],
                            start=(kt == 0), stop=(kt == NT - 1))
                    nc.vector.tensor_copy(att[hs][:, 512:784], ops1[:, 0:272])
                for hs, h in streams:
                    if pair < 2:
                        dden = den_bf[b, 0][(h % 4):(h % 4) + 1, :]
                    else:
                        dden = den_bf[b, 10 + pair][hs:hs + 1, :]
                    nc.sync.dma_start(dden, att[hs][KD:KD + 1, :])
                while fillers:
                    fillers.pop(0)()

        _dn = [0]

        def emit_dummies(n, tag):
            for i in range(n):
                _dn[0] += 1
                dp = sc_psum.tile([TP, 1024], f32, tag=f"sc{_dn[0] % 2}",
                                  name=f"dmy{tag}_{i}")
                nc.tensor.matmul(dp[0:64, 0:512], wout_sb[0][:, 0:64],
                                 wout_sb[1][:, 0:512], start=True, stop=True)

        def norm_half(b, half):
            # normalize head pairs (2*half, 2*half+1) of batch b
            from concourse.alu_op_type import AluOpType
            with nc.named_scope(f"norm_b{b}_{half}"):
                dc = den_pool.tile([4, N], f32, tag="dc")
                nc.vector.tensor_copy(dc, den_bf[b, half])
                dr = den_pool.tile([4, N], f32, tag="dr")
                nc.vector.reciprocal_approx_fast(dr, dc)
                db = den_pool.tile([4, N], bf16, tag="db")
                nc.vector.tensor_copy(db, dr)
                for pp in range(2):
                    pair = 2 * half + pp
                    for hs in range(2):
                        h = 2 * pair + hs
                        r0 = hs * 64
                        rb = rb_pool.tile([KD, N], bf16, tag="rb")
                        nc.sync.dma_start(
                            rb, db[2 * pp + hs:2 * pp + hs + 1, None, :]
                            .broadcast_to([1, KD, N]))
                        nc.vector.tensor_tensor(
                            attnT[b, pair][r0:r0 + 64, :],
                            attn_sb[b, h][0:KD, :], rb,
                            AluOpType.mult)

        def norm_pair(b, pair):
            from concourse.alu_op_type import AluOpType
            with nc.named_scope(f"normp_b{b}_{pair}"):
                dc = den_pool.tile([2, N], f32, tag="dcp")
                nc.vector.tensor_copy(dc, den_bf[b, 10 + pair])
                dr = den_pool.tile([2, N], f32, tag="drp")
                nc.vector.reciprocal_approx_fast(dr, dc)
                db = den_pool.tile([2, N], bf16, tag="dbp")
                nc.vector.tensor_copy(db, dr)
                for hs in range(2):
                    h = 2 * pair + hs
                    r0 = hs * 64
                    rb = rb_pool.tile([KD, N], bf16, tag="rb")
                    nc.sync.dma_start(
                        rb, db[hs:hs + 1, None, :].broadcast_to([1, KD, N]))
                    nc.vector.tensor_tensor(
                        attnT[b, pair][r0:r0 + 64, :],
                        attn_sb[b, h][0:KD, :], rb,
                        AluOpType.mult)

        # ---- schedule (b-major; fillers keep the PE warm during attention) --
        emit_xt(0)
        with nc.named_scope("qkv_early_b0"):
            emit_qk_tile(0, 0)
            emit_qk_tile(0, 4)
            emit_v_unit(0, 0)
            emit_v_unit(0, 1)
        attention(0, 0, [(lambda t=t: emit_v_unit(0, t)) for t in range(2, 7)]
                  + [lambda: emit_qk_tile(0, 1), lambda: emit_qk_tile(0, 5)])
        load_wout()
        emit_xt(1)
        attention(0, 1, [lambda: emit_qk_tile(0, 2), lambda: emit_qk_tile(0, 6),
                         lambda: emit_v_unit(1, 0), lambda: emit_v_unit(1, 1),
                         lambda: emit_v_unit(1, 2)])
        norm_half(0, 0)
        attention(0, 2, [lambda: emit_qk_tile(0, 3), lambda: emit_qk_tile(0, 7)]
                  + [(lambda t=t: emit_v_unit(1, t)) for t in range(3, 7)])
        norm_pair(0, 2)
        attention(0, 3, [lambda: emit_qk_tile(1, 0), lambda: emit_qk_tile(1, 4),
                         lambda: emit_qk_tile(1, 1), lambda: emit_qk_tile(1, 5)])
        norm_pair(0, 3)
        attention(1, 0, [lambda: emit_out_unit(0, 0), lambda: emit_out_unit(0, 1),
                         lambda: emit_qk_tile(1, 2)])
        attention(1, 1, [lambda: emit_out_unit(0, 2), lambda: emit_out_unit(0, 3),
                         lambda: emit_qk_tile(1, 6)])
        norm_half(1, 0)
        attention(1, 2, [lambda: emit_out_unit(0, 4), lambda: emit_out_unit(0, 5),
                         lambda: emit_qk_tile(1, 3), lambda: emit_qk_tile(1, 7),
                         lambda: emit_out_unit(0, 6)])
        norm_pair(1, 2)
        attention(1, 3, [])
        emit_dummies(6, "t0")
        norm_pair(1, 3)
        emit_dummies(6, "t1")
        with nc.named_scope("proj_b1"):
            for t in range(NT):
                emit_out_unit(1, t)
                emit_dummies(1, f"t2_{t}")

    nc.compile()
    _CACHE['nc'] = nc
    return nc


def host_prep(x, w_qkv, pos_table, w_out):
    x = np.asarray(x, np.float32).reshape(B, N, C)
    wq = np.array(np.asarray(w_qkv, np.float32), copy=True)
    wq[:, :C] *= np.float32(1.0 / np.sqrt(KD))
    wq_bf = wq.astype(ml_dtypes.bfloat16)
    idx = _rel_index()
    biasT = np.ascontiguousarray(np.exp(
        np.asarray(pos_table, np.float32)[:, idx].transpose(0, 2, 1)
    )).astype(ml_dtypes.bfloat16)
    wout = np.ascontiguousarray(np.asarray(w_out, np.float32)).astype(
        ml_dtypes.bfloat16)
    in_maps = []
    for c in range(NCORES):
        xT = np.ascontiguousarray(
            x[c * BPC:(c + 1) * BPC].transpose(0, 2, 1)).astype(
                ml_dtypes.bfloat16)  # [2, 512, 784]
        in_maps.append({"xT": xT, "wqkv": wq_bf, "wout": wout, "biasT": biasT})
    return in_maps


def run(in_maps, trace=False, trace_cores=None):
    import concourse.bass_utils as bass_utils
    nc = build_nc()
    return bass_utils.run_bass_kernel_spmd(
        nc, in_maps, core_ids=list(range(NCORES)),
        trace=trace, trace_cores=trace_cores)


def kernel(x, w_qkv, pos_table, w_out):
    in_maps = host_prep(x, w_qkv, pos_table, w_out)
    res = run(in_maps)
    out = np.stack([np.asarray(r["out"], np.float32) for r in res.results])
    return np.ascontiguousarray(out.reshape(B, HH, WW, C))

